# revision 20
# baseline (speedup 1.0000x reference)
"""3-layer GAT on 8 Trainium2 NeuronCores (Bass/Tile).

Sharding: dst-node data parallel, 1250 nodes/core (padded 1280 = 10 tiles).

v5 vs the fp16 baseline:
- L1 has no replicated dense phase / 21 MB DRAM table: per-edge source
  features come from a 256-B transpose-mode dma_gather (<=768 idxs per
  instruction -- larger transpose gathers fault) out of a [10240 x 128] fp16
  x table, and h|e_src are computed per 128-edge chunk on the PE.  The chunk
  PSUM is drained unscaled to fp16 on ACT (frees the bank immediately), then
  scaled in place by the attention weights on DVE once e_dst arrives.
- L2/L3 tables store h in fp8e4 (e_src stays fp16 at a fixed byte offset),
  shrinking gather rows 2304 B -> 1280/768 B.  The DVE scale reads fp8 and
  writes the scaled fp16 rows the aggregation matmuls consume.  Host numpy
  validation: fp8 h/w quantization gives ~6e-3 final rel err vs 2e-2 gate.
- Tables are exchanged with per-tile-pair fp8 AllGathers (half the bytes of
  the fp16 baseline) that overlap the edge phase.

Feature columns are interleaved (c*H + h) so per-head scaling is a single
stride-0-middle-broadcast DVE multiply; weights are permuted on host.
Matmuls accumulate fp32 in PSUM; denominators ride as extra columns (L3) or
a separate tiny PSUM accumulation (L1/L2).
"""
import sys

sys.path.insert(0, "/opt/trn_rl_repo")

import numpy as np
import ml_dtypes

NCORES, N, NPC, NPAD, T, P = 8, 10000, 1250, 1280, 10, 128
R = 10240            # table rows (pair-major: see _trow)
GB = 9               # chunks per gather batch (L2/L3)
GB1 = 6              # chunks per transpose-gather batch (L1)
SHIFTS = (3.5, 1.25, 1.0)

F16 = np.float16
F8 = ml_dtypes.float8_e4m3

# table row width in fp16 elems (gather elem bytes must be mult of 256)
L2_TBC, L2_ESOFF = 1152, 1024   # 1024 f16 h | 4 f16 es | pad (2304 B)
L3_TBC, L3_ESOFF = 768, 726    # 726 f16 h | 6 f16 es | pad (1536 B)


def _trow(g):
    # Row layout matches the 2-tile AllGather concat: [tile-pair][rank][tile][p]
    q, r = g // NPC, g % NPC
    t, p = r // P, r % P
    return (t // 2) * 2048 + q * 256 + (t % 2) * P + p


def _wrap_idx(idx):
    """[n] -> [128, n//16] int16 (wrapped in 16 partitions, replicated 8x)."""
    blk = idx.astype(np.int16).reshape(-1, 16).T.copy()
    return np.tile(blk, (8, 1))


def preprocess(inputs):
    x = np.asarray(inputs["x"], np.float32)
    ei = np.asarray(inputs["edge_index"])
    src = np.concatenate([ei[0], np.arange(N)]).astype(np.int64)
    dst = np.concatenate([ei[1], np.arange(N)]).astype(np.int64)
    order = np.argsort(dst, kind="stable")
    src, dst = src[order], dst[order]

    # per-(core,tile) edge lists and uniform chunk grid
    per = []
    K_T = 0
    for c in range(NCORES):
        m = (dst >= c * NPC) & (dst < (c + 1) * NPC)
        s, d = src[m], dst[m] - c * NPC
        tiles = []
        for t in range(T):
            mt = (d >= t * P) & (d < (t + 1) * P)
            tiles.append((s[mt], d[mt] - t * P))
            K_T = max(K_T, (int(mt.sum()) + P - 1) // P)
        per.append(tiles)
    NCH = T * K_T

    gidx, sts, st2s, xlocs = [], [], [], []
    for c in range(NCORES):
        ss = np.zeros((T, K_T * P), np.int64)
        dd = np.zeros((T, K_T * P), np.int64)
        vv = np.zeros((T, K_T * P), bool)
        for t in range(T):
            s, d = per[c][t]
            n = len(s)
            ss[t, :n], dd[t, :n], vv[t, :n] = s, d, True
        rows = _trow(ss).reshape(-1)
        gidx.append(_wrap_idx(rows))
        S = np.zeros((T, K_T * P, P), np.float32)
        ar = np.arange(K_T * P)
        for t in range(T):
            sl = ar[vv[t]]
            S[t, sl, dd[t][vv[t]]] = 1.0
        S = S.reshape(NCH, P, P)
        sts.append(np.ascontiguousarray(
            S.transpose(1, 0, 2).reshape(P, NCH * P)).astype(F16))
        st2s.append(np.ascontiguousarray(
            S.transpose(2, 0, 1).reshape(P, NCH * P)).astype(F16))
        # local x transposed [128 feat x NPAD] (L1 local e_dst matmuls)
        xl = np.zeros((NPAD, 128), np.float32)
        xl[:NPC, :50] = x[c * NPC:(c + 1) * NPC]
        xlocs.append(np.ascontiguousarray(xl.T).astype(F16))

    # weights (shared), interleaved feature order (c*H + h)
    def w_aug(W, a_s, a_d, fin_pad, prev_hc=None):
        W = np.asarray(W, np.float32)
        H, C = a_s.shape
        F = W.shape[1]
        if prev_hc is not None:
            Hp, Cp = prev_hc
            perm = (np.arange(Cp)[:, None] + np.arange(Hp)[None, :] * Cp).reshape(-1)
            W = W[:, perm]
        Wp = W.reshape(H, C, F)
        Wi = np.transpose(Wp, (2, 1, 0)).reshape(F, C * H)
        es = np.einsum("hcf,hc->fh", Wp, np.asarray(a_s, np.float32))
        ed = np.einsum("hcf,hc->fh", Wp, np.asarray(a_d, np.float32))
        out = np.concatenate([Wi, es, ed], 1)
        return np.concatenate(
            [out, np.zeros((fin_pad - F, out.shape[1]), np.float32)], 0
        ).astype(F16)

    w1 = w_aug(inputs["W1"], np.asarray(inputs["as1"]), np.asarray(inputs["ad1"]), 128)
    w2 = w_aug(inputs["W2"], np.asarray(inputs["as2"]), np.asarray(inputs["ad2"]), 1024,
               prev_hc=(4, 256))
    w3 = w_aug(inputs["W3"], np.asarray(inputs["as3"]), np.asarray(inputs["ad3"]), 1024,
               prev_hc=(4, 256))

    # x table for L1 transpose-gather: [R, 128] fp16, row = _trow(node)
    xt = np.zeros((R, 128), np.float32)
    xt[_trow(np.arange(N)), :50] = x
    x1T = xt.astype(F16)

    shared = {"x1T": x1T, "w1": w1, "w2": w2, "w3": w3}
    percore = [
        {"gidx": gidx[c], "st": sts[c], "st2": st2s[c], "x1lsT": xlocs[c]}
        for c in range(NCORES)
    ]
    return K_T, shared, percore


_CACHE = {}


def build_program(K_T):
    import concourse.bacc as bacc
    import concourse.mybir as mybir
    import concourse.tile as tile

    dt = mybir.dt
    AF = mybir.ActivationFunctionType
    AL = mybir.AluOpType
    NCH = T * K_T

    nc = bacc.Bacc("TRN2", target_bir_lowering=False, debug=False, num_devices=NCORES)

    def register_const(val):
        t = nc.alloc_sbuf_tensor(f"constx-{val}", [128, 1], dt.float32)
        nc.gpsimd.memset(t.ap(), val)
        nc.const_aps.aps[(dt.float32, val)] = t.ap()

    for s in SHIFTS:
        if (dt.float32, -s) not in nc.const_aps.aps:
            register_const(-s)
    nc.all_engine_barrier()

    x1T = nc.dram_tensor("x1T", [R, 128], dt.float16, kind="ExternalInput")
    w1 = nc.dram_tensor("w1", [128, 1032], dt.float16, kind="ExternalInput")
    w2 = nc.dram_tensor("w2", [1024, 1032], dt.float16, kind="ExternalInput")
    w3 = nc.dram_tensor("w3", [1024, 738], dt.float16, kind="ExternalInput")
    gidx = nc.dram_tensor("gidx", [128, NCH * 8], dt.int16, kind="ExternalInput")
    st = nc.dram_tensor("st", [128, NCH * 128], dt.float16, kind="ExternalInput")
    st2 = nc.dram_tensor("st2", [128, NCH * 128], dt.float16, kind="ExternalInput")
    x1lsT = nc.dram_tensor("x1lsT", [128, NPAD], dt.float16, kind="ExternalInput")
    out = nc.dram_tensor("out", [NPAD, 121], dt.float32, kind="ExternalOutput")

    tableB = nc.dram_tensor("tableB", [R, L2_TBC], dt.float16, addr_space="Shared")
    table3 = nc.dram_tensor("table3", [R, L3_TBC], dt.float16, addr_space="Shared")
    bounceB = nc.dram_tensor("bounceB", [NPAD, L2_TBC], dt.float16)
    bounce3 = nc.dram_tensor("bounce3", [NPAD, L3_TBC], dt.float16)

    RG = [list(range(NCORES))]

    with tile.TileContext(nc) as tc:
        from concourse.masks import make_identity

        with (
            tc.tile_pool(name="per", bufs=1) as per,
            tc.tile_pool(name="gp", bufs=2) as gp,
            tc.tile_pool(name="dp", bufs=2) as dp,
            tc.tile_pool(name="sp", bufs=2) as sp,
            tc.tile_pool(name="wp", bufs=2) as wp,
            tc.tile_pool(name="ep", bufs=2) as ep,
            tc.tile_pool(name="ps", bufs=1, space="PSUM") as ps,
        ):
            # persistent loads
            w1s = per.tile([128, 1032], dt.float16)
            nc.sync.dma_start(w1s[:], w1[:])
            w2s = per.tile([128, 8, 1032], dt.float16)
            nc.sync.dma_start(w2s[:], w2.ap().rearrange("(a p) n -> p a n", p=128))
            w3s = per.tile([128, 8, 738], dt.float16)
            nc.sync.dma_start(w3s[:], w3.ap().rearrange("(a p) n -> p a n", p=128))
            g1i = per.tile([128, NCH * 8], dt.int16)
            nc.sync.dma_start(g1i[:], gidx[:])
            x1ls = per.tile([128, NPAD], dt.float16)
            nc.sync.dma_start(x1ls[:], x1lsT[:])
            edl1 = per.tile([128, T, 8], dt.float16)
            edlB = per.tile([128, T, 8], dt.float16)
            edl3 = per.tile([128, T, 8], dt.float16)
            idf16 = per.tile([128, 128], dt.float16)
            make_identity(nc, idf16[:])
            xTs = per.tile([128, 8, NPAD], dt.float16)
            xres = per.tile([128, T, 1024], dt.float16)

            # L1 local e_dst (tiny matmuls from local x)
            for t in range(T):
                pse = ps.tile([128, 128], dt.float32, tag="DE", name="pse")
                nc.tensor.matmul(
                    pse[:, 64:72], x1ls[:, t * P:(t + 1) * P], w1s[:, 1024:1032],
                    start=True, stop=True,
                )
                nc.scalar.copy(edl1[:, t, 0:4], pse[:, 68:72])

            # ---------- uniform edge phase ----------
            def edge_phase(l1, table, TBC, ESOFF, H, C, shift, edl, epi_fn):
                DO = H * C
                NW = DO + H          # with-denominator width (L3 rides in psB)
                sepC = NW > 1024     # L1/L2: denominator in separate psC
                for t in range(T):
                    psA = ps.tile([128, 512], dt.float32, tag="A")
                    psB = ps.tile([128, 512], dt.float32, tag="B")
                    if sepC:
                        psC = ps.tile([128, 64], dt.float32, tag="C")
                    else:
                        psC = None
                    if l1:
                        batches = [(k, min(k + GB1, K_T)) for k in range(0, K_T, GB1)]
                    else:
                        batches = [(0, GB), (GB, K_T)]
                    for (k0, k1) in batches:
                        nb = k1 - k0
                        off8 = (t * K_T + k0) * 8
                        c0, c1 = (t * K_T + k0) * 128, (t * K_T + k1) * 128
                        s = sp.tile([128, nb * 128], dt.float16, tag="s")
                        nc.sync.dma_start(s[:], st[:, c0:c1])
                        s2 = dp.tile([128, nb * 128], dt.float16, tag="s2")
                        nc.sync.dma_start(s2[:], st2[:, c0:c1])
                        if l1:
                            xg = gp.tile([128, 1, nb * 128], dt.float16, tag="xg")
                            nc.gpsimd.dma_gather(
                                xg[:], x1T.ap(), g1i[:, off8:off8 + nb * 8],
                                num_idxs=nb * 128, num_idxs_reg=nb * 128,
                                elem_size=128, transpose=True,
                            )
                            g16 = gp.tile([128, GB1, 1024], dt.float16,
                                          tag="g16")
                            de = ps.tile([128, 128], dt.float32, tag="DE", name="de")
                            # e_src pass first so the w chain never waits on
                            # the h denses below
                            for k in range(nb):
                                nc.tensor.matmul(
                                    de[:, 64 + k * 4:64 + k * 4 + 4],
                                    xg[:, 0, k * 128:(k + 1) * 128],
                                    w1s[:, 1024:1028], start=True, stop=True,
                                )
                            est = wp.tile([128, nb, 4], dt.float16, tag="est")
                            nc.vector.tensor_copy(
                                est[:],
                                de[:, 64:64 + nb * 4].rearrange(
                                    "p (b h) -> p b h", h=4
                                ),
                            )
                            es3 = est[:]
                        else:
                            g16 = gp.tile([128, GB, TBC], dt.float16, tag="g")
                            nc.gpsimd.dma_gather(
                                g16[:, 0:nb], table.ap(),
                                g1i[:, off8:off8 + nb * 8],
                                num_idxs=nb * 128, num_idxs_reg=nb * 128,
                                elem_size=TBC, single_packet=False,
                            )
                            es3 = g16[:, 0:nb, ESOFF:ESOFF + H]
                        # e_dst broadcast to edges
                        if l1:
                            psD = de
                        else:
                            psD = ps.tile([128, 128], dt.float32, tag="DE",
                                          name="psD")
                        for k in range(nb):
                            nc.tensor.matmul(
                                psD[:, k * H:(k + 1) * H],
                                s2[:, k * 128:(k + 1) * 128],
                                edl[:, t, 0:H],
                                start=True, stop=True,
                            )
                        # w = exp(leaky(es + ed) - shift)
                        ew = wp.tile([128, nb * H], dt.float32, tag="ew")
                        ew3 = ew[:].rearrange("p (b h) -> p b h", h=H)
                        nc.vector.tensor_tensor(
                            ew3, es3,
                            psD[:, 0:nb * H].rearrange("p (b h) -> p b h", h=H),
                            op=AL.add,
                        )
                        nc.vector.scalar_tensor_tensor(
                            ew[:], ew[:], 0.2, ew[:], op0=AL.mult, op1=AL.max
                        )
                        w16 = wp.tile([128, nb, H], dt.float16, tag="w16")
                        nc.scalar.activation(w16[:], ew3, AF.Exp, bias=-shift)
                        if l1:
                            # w16 is ready before the denses, so the drain
                            # IS the scale: one DVE op per chunk
                            for k in range(nb):
                                lhsT = xg[:, 0, k * 128:(k + 1) * 128]
                                psH = ps.tile(
                                    [128, 1024], dt.float32, tag="H", name="psH",
                                    bufs=2,
                                )
                                nc.tensor.matmul(
                                    psH[:, 0:512], lhsT, w1s[:, 0:512],
                                    start=True, stop=True,
                                )
                                nc.tensor.matmul(
                                    psH[:, 512:1024], lhsT, w1s[:, 512:1024],
                                    start=True, stop=True,
                                )
                                wk = w16[:, k, :].rearrange(
                                    "p (o h) -> p o h", o=1
                                ).to_broadcast([128, C, H])
                                nc.vector.tensor_tensor(
                                    g16[:, k, 0:DO].rearrange(
                                        "p (c h) -> p c h", h=H
                                    ),
                                    psH[:].rearrange("p (c h) -> p c h", h=H),
                                    wk, op=AL.mult,
                                )
                        else:
                            # scale rows per head in place (stride-0 middle)
                            for k in range(nb):
                                wk = w16[:, k, :].rearrange(
                                    "p (o h) -> p o h", o=1
                                ).to_broadcast([128, C, H])
                                nc.vector.tensor_tensor(
                                    g16[:, k, 0:DO].rearrange(
                                        "p (c h) -> p c h", h=H
                                    ),
                                    g16[:, k, 0:DO].rearrange(
                                        "p (c h) -> p c h", h=H
                                    ),
                                    wk, op=AL.mult,
                                )
                        if not sepC:
                            # denominator rides in psB: copy w into g16 tail
                            nc.vector.tensor_copy(
                                g16[:, 0:nb, DO:DO + H], w16[:]
                            )
                        # aggregation matmuls
                        for k in range(nb):
                            kk = k0 + k
                            fl, ll = kk == 0, kk == K_T - 1
                            sT = s[:, k * 128:(k + 1) * 128]
                            bw = min(512, NW - 512)
                            nc.tensor.matmul(
                                psA[:], sT, g16[:, k, 0:512], start=fl, stop=ll
                            )
                            nc.tensor.matmul(
                                psB[:, :bw], sT, g16[:, k, 512:512 + bw],
                                start=fl, stop=ll,
                            )
                            if sepC:
                                nc.tensor.matmul(
                                    psC[:, :H], sT, w16[:, k, :],
                                    start=fl, stop=ll,
                                )
                    epi_fn(t, psA, psB, psC)

            # ---------- epilogues ----------
            def normalize12(t, psA, psB, psC, H, C):
                dn = wp.tile([128, H], dt.float32, tag="dn")
                nc.vector.tensor_scalar_max(dn[:], psC[:, :H], 1e-16)
                r = wp.tile([128, H], dt.float32, tag="r")
                nc.vector.reciprocal(r[:], dn[:])
                xt = ep.tile([128, 1024], dt.float16, tag="xt")
                rb = r[:].rearrange("p (o h) -> p o h", o=1).to_broadcast(
                    [128, 128, H]
                )
                for half, pst in ((0, psA), (1, psB)):
                    nc.vector.tensor_tensor(
                        xt[:, half * 512:(half + 1) * 512].rearrange(
                            "p (c h) -> p c h", h=H
                        ),
                        pst[:].rearrange("p (c h) -> p c h", h=H),
                        rb, op=AL.mult,
                    )
                return xt

            def elu_into(xt, dest):
                # elu(x) = relu(x) + exp(-relu(-x)) - 1, keeping DVE light
                neg = ep.tile([128, 1024], dt.float16, tag="neg")
                nc.scalar.activation(neg[:], xt[:], AF.Relu, scale=-1.0)
                en = ep.tile([128, 1024], dt.float16, tag="en")
                nc.scalar.activation(en[:], neg[:], AF.Exp, scale=-1.0)
                a = ep.tile([128, 1024], dt.float16, tag="a")
                nc.scalar.activation(a[:], xt[:], AF.Relu)
                nc.vector.scalar_tensor_tensor(
                    dest, a[:], -1.0, en[:], op0=AL.add, op1=AL.add
                )

            def dense_next(t, xsrc, w_sb, DO, H, ESOFF, bounce, edlN, tableN):
                # PE-transpose x tile into xTs
                tp = ps.tile([128, 8, 128], dt.float16, tag="H", name="tp",
                             bufs=2)
                for fb in range(8):
                    nc.tensor.transpose(
                        tp[:, fb, :], xsrc[:, fb * 128:(fb + 1) * 128], idf16[:]
                    )
                nc.scalar.copy(xTs[:, :, t * P:(t + 1) * P], tp[:])
                used = ESOFF + H
                tabst = ep.tile([128, used], dt.float16, tag="tabst", bufs=3)
                psE2 = ps.tile([128, 128], dt.float32, tag="DE", name="psE2")
                for k in range(8):
                    nc.tensor.matmul(
                        psE2[:, 64:64 + 2 * H],
                        xTs[:, k, t * P:(t + 1) * P],
                        w_sb[:, k, DO:DO + 2 * H],
                        start=(k == 0), stop=(k == 7),
                    )
                nc.vector.tensor_copy(
                    tabst[:, ESOFF:ESOFF + H], psE2[:, 64:64 + H]
                )
                nc.vector.tensor_copy(edlN[:, t, 0:H], psE2[:, 64 + H:64 + 2 * H])
                psN = ps.tile([128, 1024], dt.float32, tag="H", name="psN",
                              bufs=2)
                for si, (o, wd) in enumerate(((0, 512), (512, DO - 512))):
                    for k in range(8):
                        nc.tensor.matmul(
                            psN[:, o:o + wd],
                            xTs[:, k, t * P:(t + 1) * P],
                            w_sb[:, k, o:o + wd],
                            start=(k == 0), stop=(k == 7),
                        )
                    nc.scalar.copy(tabst[:, o:o + wd], psN[:, o:o + wd])
                nc.sync.dma_start(bounce[t * P:(t + 1) * P, 0:used], tabst[:])
                if t % 2 == 1:
                    nc.gpsimd.collective_compute(
                        "AllGather", mybir.AluOpType.bypass,
                        ins=[bounce[(t - 1) * P:(t + 1) * P, :].opt()],
                        outs=[tableN[(t - 1) * 1024:(t + 1) * 1024, :].opt()],
                        replica_groups=RG,
                    )

            # L1 edge: epilogue computes x2 (elu), stores to xres, builds L2 table
            def epi1(t, psA, psB, psC):
                xt = normalize12(t, psA, psB, psC, 4, 256)
                elu_into(xt, xres[:, t, :])
                dense_next(
                    t, xres[:, t, :], w2s, 1024, 4, L2_ESOFF,
                    bounceB, edlB, tableB,
                )

            edge_phase(True, None, 0, 0, 4, 256, SHIFTS[0], edl1, epi1)

            # L2 edge: epilogue x3 = elu(norm + x2), builds L3 table
            def epi2(t, psA, psB, psC):
                xt = normalize12(t, psA, psB, psC, 4, 256)
                nc.vector.tensor_add(xt[:], xt[:], xres[:, t, :])
                x3 = ep.tile([128, 1024], dt.float16, tag="x3")
                elu_into(xt, x3[:])
                dense_next(
                    t, x3[:], w3s, 726, 6, L3_ESOFF,
                    bounce3, edl3, table3,
                )

            edge_phase(False, tableB, L2_TBC, L2_ESOFF, 4, 256, SHIFTS[1], edlB, epi2)

            # L3 edge: final epilogue (mean over 6 heads, sigmoid)
            def epi3(t, psA, psB, psC):
                t732 = ep.tile([128, 732], dt.float32, tag="t732")
                nc.scalar.copy(t732[:, 0:512], psA[:])
                nc.vector.tensor_copy(t732[:, 512:732], psB[:, 0:220])
                dn = wp.tile([128, 6], dt.float32, tag="dn3")
                nc.vector.tensor_scalar_max(dn[:], t732[:, 726:732], 1e-16)
                r = wp.tile([128, 6], dt.float32, tag="r3")
                nc.vector.reciprocal(r[:], dn[:])
                r6 = wp.tile([128, 6], dt.float32, tag="r6")
                nc.vector.tensor_scalar_mul(r6[:], r[:], 1.0 / 6.0)
                tmp = ep.tile([128, 726], dt.float32, tag="tmp726")
                nc.vector.tensor_tensor(
                    tmp[:].rearrange("p (c h) -> p c h", h=6),
                    t732[:, 0:726].rearrange("p (c h) -> p c h", h=6),
                    r6[:].rearrange("p (o h) -> p o h", o=1).to_broadcast(
                        [128, 121, 6]
                    ),
                    op=AL.mult,
                )
                o121 = ep.tile([128, 121], dt.float32, tag="o121")
                nc.vector.reduce_sum(
                    o121[:], tmp[:].rearrange("p (c h) -> p c h", h=6),
                    mybir.AxisListType.X,
                )
                nc.scalar.activation(o121[:], o121[:], AF.Sigmoid)
                nc.sync.dma_start(out[t * P:(t + 1) * P, :], o121[:])

            edge_phase(False, table3, L3_TBC, L3_ESOFF, 6, 121, SHIFTS[2], edl3, epi3)

    nc.compile()
    return nc


def run(inputs, trace=False, tmpdir=None):
    from concourse.bass_utils import run_bass_kernel_spmd

    K_T, shared, percore = preprocess(inputs)
    key = K_T
    if key not in _CACHE:
        _CACHE[key] = build_program(K_T)
    nc = _CACHE[key]
    in_maps = [{**shared, **percore[c]} for c in range(NCORES)]
    if trace:
        import types

        try:
            import antenv.axon_hooks  # noqa: F401
        except ImportError:
            from trn_agent_boot.trn_boot import _ntff_profile_via_ctypes

            m = types.ModuleType("antenv.axon_hooks")
            hook = _ntff_profile_via_ctypes("/opt/axon/libaxon_pjrt.so")
            m.get_axon_ntff_profile_hook = lambda: hook
            sys.modules["antenv.axon_hooks"] = m
    try:
        res = run_bass_kernel_spmd(
            nc, in_maps, list(range(NCORES)), trace=trace, tmpdir=tmpdir
        )
    except Exception:
        # A crashed earlier run can leave the device wedged; one retry
        # normally clears it (nrt reopen).
        import time

        time.sleep(2)
        res = run_bass_kernel_spmd(
            nc, in_maps, list(range(NCORES)), trace=trace, tmpdir=tmpdir
        )
    outs = []
    for c in range(NCORES):
        outs.append(res.results[c]["out"][:NPC])
    full = np.concatenate(outs, 0).astype(np.float32)
    return full, res


def kernel(**inputs):
    full, _ = run(inputs)
    return full


# revision 21
# speedup vs baseline: 1.0841x; 1.0841x over previous
"""3-layer GAT on 8 Trainium2 NeuronCores (Bass/Tile).

Sharding: dst-node data parallel, 1250 nodes/core (padded 1280 = 10 tiles).

v5 vs the fp16 baseline:
- L1 has no replicated dense phase / 21 MB DRAM table: per-edge source
  features come from a 256-B transpose-mode dma_gather (<=768 idxs per
  instruction -- larger transpose gathers fault) out of a [10240 x 128] fp16
  x table, and h|e_src are computed per 128-edge chunk on the PE.  The chunk
  PSUM is drained unscaled to fp16 on ACT (frees the bank immediately), then
  scaled in place by the attention weights on DVE once e_dst arrives.
- L2/L3 tables store h in fp8e4 (e_src stays fp16 at a fixed byte offset),
  shrinking gather rows 2304 B -> 1280/768 B.  The DVE scale reads fp8 and
  writes the scaled fp16 rows the aggregation matmuls consume.  Host numpy
  validation: fp8 h/w quantization gives ~6e-3 final rel err vs 2e-2 gate.
- Tables are exchanged with per-tile-pair fp8 AllGathers (half the bytes of
  the fp16 baseline) that overlap the edge phase.

Feature columns are interleaved (c*H + h) so per-head scaling is a single
stride-0-middle-broadcast DVE multiply; weights are permuted on host.
Matmuls accumulate fp32 in PSUM; denominators ride as extra columns (L3) or
a separate tiny PSUM accumulation (L1/L2).
"""
import sys

sys.path.insert(0, "/opt/trn_rl_repo")

import numpy as np
import ml_dtypes

NCORES, N, NPC, NPAD, T, P = 8, 10000, 1250, 1280, 10, 128
R = 10240            # table rows (pair-major: see _trow)
GB = 9               # chunks per gather batch (L2/L3)
GB1 = 6              # chunks per transpose-gather batch (L1)
SHIFTS = (3.5, 1.25, 1.0)

F16 = np.float16
F8 = ml_dtypes.float8_e4m3

# table row width in fp16 elems (gather elem bytes must be mult of 256)
L2_TBC, L2_ESOFF = 1152, 1024   # 1024 f16 h | 4 f16 es | pad (2304 B)
L3_TBC, L3_ESOFF = 768, 726    # 726 f16 h | 6 f16 es | pad (1536 B)


def _trow(g):
    # Row layout matches the 2-tile AllGather concat: [tile-pair][rank][tile][p]
    q, r = g // NPC, g % NPC
    t, p = r // P, r % P
    return (t // 2) * 2048 + q * 256 + (t % 2) * P + p


def _wrap_idx(idx):
    """[n] -> [128, n//16] int16 (wrapped in 16 partitions, replicated 8x)."""
    blk = idx.astype(np.int16).reshape(-1, 16).T.copy()
    return np.tile(blk, (8, 1))


def preprocess(inputs):
    x = np.asarray(inputs["x"], np.float32)
    ei = np.asarray(inputs["edge_index"])
    src = np.concatenate([ei[0], np.arange(N)]).astype(np.int64)
    dst = np.concatenate([ei[1], np.arange(N)]).astype(np.int64)
    order = np.argsort(dst, kind="stable")
    src, dst = src[order], dst[order]

    # per-(core,tile) edge lists and uniform chunk grid
    per = []
    K_T = 0
    for c in range(NCORES):
        m = (dst >= c * NPC) & (dst < (c + 1) * NPC)
        s, d = src[m], dst[m] - c * NPC
        tiles = []
        for t in range(T):
            mt = (d >= t * P) & (d < (t + 1) * P)
            tiles.append((s[mt], d[mt] - t * P))
            K_T = max(K_T, (int(mt.sum()) + P - 1) // P)
        per.append(tiles)
    NCH = T * K_T

    gidx, sts, st2s, xlocs = [], [], [], []
    for c in range(NCORES):
        ss = np.zeros((T, K_T * P), np.int64)
        dd = np.zeros((T, K_T * P), np.int64)
        vv = np.zeros((T, K_T * P), bool)
        for t in range(T):
            s, d = per[c][t]
            n = len(s)
            ss[t, :n], dd[t, :n], vv[t, :n] = s, d, True
        rows = _trow(ss).reshape(-1)
        gidx.append(_wrap_idx(rows))
        S = np.zeros((T, K_T * P, P), np.float32)
        ar = np.arange(K_T * P)
        for t in range(T):
            sl = ar[vv[t]]
            S[t, sl, dd[t][vv[t]]] = 1.0
        S = S.reshape(NCH, P, P)
        sts.append(np.ascontiguousarray(
            S.transpose(1, 0, 2).reshape(P, NCH * P)).astype(F16))
        st2s.append(np.ascontiguousarray(
            S.transpose(2, 0, 1).reshape(P, NCH * P)).astype(F16))
        # local x transposed [128 feat x NPAD] (L1 local e_dst matmuls)
        xl = np.zeros((NPAD, 128), np.float32)
        xl[:NPC, :50] = x[c * NPC:(c + 1) * NPC]
        xlocs.append(np.ascontiguousarray(xl.T).astype(F16))

    # weights (shared), interleaved feature order (c*H + h)
    def w_aug(W, a_s, a_d, fin_pad, prev_hc=None):
        W = np.asarray(W, np.float32)
        H, C = a_s.shape
        F = W.shape[1]
        if prev_hc is not None:
            Hp, Cp = prev_hc
            perm = (np.arange(Cp)[:, None] + np.arange(Hp)[None, :] * Cp).reshape(-1)
            W = W[:, perm]
        Wp = W.reshape(H, C, F)
        Wi = np.transpose(Wp, (2, 1, 0)).reshape(F, C * H)
        es = np.einsum("hcf,hc->fh", Wp, np.asarray(a_s, np.float32))
        ed = np.einsum("hcf,hc->fh", Wp, np.asarray(a_d, np.float32))
        out = np.concatenate([Wi, es, ed], 1)
        return np.concatenate(
            [out, np.zeros((fin_pad - F, out.shape[1]), np.float32)], 0
        ).astype(F16)

    w1 = w_aug(inputs["W1"], np.asarray(inputs["as1"]), np.asarray(inputs["ad1"]), 128)
    w2 = w_aug(inputs["W2"], np.asarray(inputs["as2"]), np.asarray(inputs["ad2"]), 1024,
               prev_hc=(4, 256))
    w3 = w_aug(inputs["W3"], np.asarray(inputs["as3"]), np.asarray(inputs["ad3"]), 1024,
               prev_hc=(4, 256))

    # x table for L1 transpose-gather: [R, 128] fp16, row = _trow(node)
    xt = np.zeros((R, 128), np.float32)
    xt[_trow(np.arange(N)), :50] = x
    x1T = xt.astype(F16)

    shared = {"x1T": x1T, "w1": w1, "w2": w2, "w3": w3}
    percore = [
        {"gidx": gidx[c], "st": sts[c], "st2": st2s[c], "x1lsT": xlocs[c]}
        for c in range(NCORES)
    ]
    return K_T, shared, percore


_CACHE = {}


def build_program(K_T):
    import concourse.bacc as bacc
    import concourse.mybir as mybir
    import concourse.tile as tile

    dt = mybir.dt
    AF = mybir.ActivationFunctionType
    AL = mybir.AluOpType
    NCH = T * K_T

    nc = bacc.Bacc("TRN2", target_bir_lowering=False, debug=False, num_devices=NCORES)

    def register_const(val):
        t = nc.alloc_sbuf_tensor(f"constx-{val}", [128, 1], dt.float32)
        nc.gpsimd.memset(t.ap(), val)
        nc.const_aps.aps[(dt.float32, val)] = t.ap()

    for s in SHIFTS:
        if (dt.float32, -s) not in nc.const_aps.aps:
            register_const(-s)
    nc.all_engine_barrier()

    x1T = nc.dram_tensor("x1T", [R, 128], dt.float16, kind="ExternalInput")
    w1 = nc.dram_tensor("w1", [128, 1032], dt.float16, kind="ExternalInput")
    w2 = nc.dram_tensor("w2", [1024, 1032], dt.float16, kind="ExternalInput")
    w3 = nc.dram_tensor("w3", [1024, 738], dt.float16, kind="ExternalInput")
    gidx = nc.dram_tensor("gidx", [128, NCH * 8], dt.int16, kind="ExternalInput")
    st = nc.dram_tensor("st", [128, NCH * 128], dt.float16, kind="ExternalInput")
    st2 = nc.dram_tensor("st2", [128, NCH * 128], dt.float16, kind="ExternalInput")
    x1lsT = nc.dram_tensor("x1lsT", [128, NPAD], dt.float16, kind="ExternalInput")
    out = nc.dram_tensor("out", [NPAD, 121], dt.float32, kind="ExternalOutput")

    tableB = nc.dram_tensor("tableB", [R, L2_TBC], dt.float16, addr_space="Shared")
    table3 = nc.dram_tensor("table3", [R, L3_TBC], dt.float16, addr_space="Shared")
    bounceB = nc.dram_tensor("bounceB", [NPAD, L2_TBC], dt.float16)
    bounce3 = nc.dram_tensor("bounce3", [NPAD, L3_TBC], dt.float16)

    RG = [list(range(NCORES))]

    with tile.TileContext(nc) as tc:
        from concourse.masks import make_identity

        with (
            tc.tile_pool(name="per", bufs=1) as per,
            tc.tile_pool(name="gp", bufs=2) as gp,
            tc.tile_pool(name="dp", bufs=2) as dp,
            tc.tile_pool(name="sp", bufs=2) as sp,
            tc.tile_pool(name="wp", bufs=2) as wp,
            tc.tile_pool(name="ep", bufs=2) as ep,
            tc.tile_pool(name="ps", bufs=1, space="PSUM") as ps,
        ):
            # persistent loads
            w1s = per.tile([128, 1032], dt.float16)
            nc.sync.dma_start(w1s[:], w1[:])
            w2s = per.tile([128, 8, 1032], dt.float16)
            nc.sync.dma_start(w2s[:], w2.ap().rearrange("(a p) n -> p a n", p=128))
            w3s = per.tile([128, 8, 738], dt.float16)
            nc.sync.dma_start(w3s[:], w3.ap().rearrange("(a p) n -> p a n", p=128))
            g1i = per.tile([128, NCH * 8], dt.int16)
            nc.sync.dma_start(g1i[:], gidx[:])
            x1ls = per.tile([128, NPAD], dt.float16)
            nc.sync.dma_start(x1ls[:], x1lsT[:])
            edl1 = per.tile([128, T, 8], dt.float16)
            edlB = per.tile([128, T, 8], dt.float16)
            edl3 = per.tile([128, T, 8], dt.float16)
            idf16 = per.tile([128, 128], dt.float16)
            make_identity(nc, idf16[:])
            xres = per.tile([128, T, 1024], dt.float16)

            # L1 local e_dst (tiny matmuls from local x)
            for t in range(T):
                pse = ps.tile([128, 128], dt.float32, tag="DE", name="pse")
                nc.tensor.matmul(
                    pse[:, 64:72], x1ls[:, t * P:(t + 1) * P], w1s[:, 1024:1032],
                    start=True, stop=True,
                )
                nc.scalar.copy(edl1[:, t, 0:4], pse[:, 68:72])

            # ---------- uniform edge phase ----------
            def edge_phase(l1, table, TBC, ESOFF, H, C, shift, edl, epi_fn):
                DO = H * C
                NW = DO + H          # with-denominator width (L3 rides in psB)
                sepC = NW > 1024     # L1/L2: denominator in separate psC
                for t in range(T):
                    psA = ps.tile([128, 512], dt.float32, tag="A")
                    psB = ps.tile([128, 512], dt.float32, tag="B")
                    if sepC:
                        psC = ps.tile([128, 64], dt.float32, tag="C")
                    else:
                        psC = None
                    if l1:
                        batches = [(k, min(k + GB1, K_T)) for k in range(0, K_T, GB1)]
                    else:
                        batches = [(0, GB), (GB, K_T)]
                    for (k0, k1) in batches:
                        nb = k1 - k0
                        off8 = (t * K_T + k0) * 8
                        c0, c1 = (t * K_T + k0) * 128, (t * K_T + k1) * 128
                        s = sp.tile([128, nb * 128], dt.float16, tag="s")
                        nc.sync.dma_start(s[:], st[:, c0:c1])
                        s2 = dp.tile([128, nb * 128], dt.float16, tag="s2")
                        nc.sync.dma_start(s2[:], st2[:, c0:c1])
                        if l1:
                            xg = gp.tile([128, 1, nb * 128], dt.float16, tag="xg")
                            nc.gpsimd.dma_gather(
                                xg[:], x1T.ap(), g1i[:, off8:off8 + nb * 8],
                                num_idxs=nb * 128, num_idxs_reg=nb * 128,
                                elem_size=128, transpose=True,
                            )
                            g16 = gp.tile([128, GB1, 1024], dt.float16,
                                          tag="g", bufs=3)
                            de = ps.tile([128, 128], dt.float32, tag="DE", name="de")
                            # e_src pass first so the w chain never waits on
                            # the h denses below
                            for k in range(nb):
                                nc.tensor.matmul(
                                    de[:, 64 + k * 4:64 + k * 4 + 4],
                                    xg[:, 0, k * 128:(k + 1) * 128],
                                    w1s[:, 1024:1028], start=True, stop=True,
                                )
                            est = wp.tile([128, nb, 4], dt.float16, tag="est")
                            nc.vector.tensor_copy(
                                est[:],
                                de[:, 64:64 + nb * 4].rearrange(
                                    "p (b h) -> p b h", h=4
                                ),
                            )
                            es3 = est[:]
                        else:
                            g16 = gp.tile([128, GB, TBC], dt.float16, tag="g",
                                          bufs=3)
                            nc.gpsimd.dma_gather(
                                g16[:, 0:nb], table.ap(),
                                g1i[:, off8:off8 + nb * 8],
                                num_idxs=nb * 128, num_idxs_reg=nb * 128,
                                elem_size=TBC, single_packet=False,
                            )
                            es3 = g16[:, 0:nb, ESOFF:ESOFF + H]
                        # e_dst broadcast to edges
                        if l1:
                            psD = de
                        else:
                            psD = ps.tile([128, 128], dt.float32, tag="DE",
                                          name="psD")
                        for k in range(nb):
                            nc.tensor.matmul(
                                psD[:, k * H:(k + 1) * H],
                                s2[:, k * 128:(k + 1) * 128],
                                edl[:, t, 0:H],
                                start=True, stop=True,
                            )
                        # w = exp(leaky(es + ed) - shift)
                        ew = wp.tile([128, nb * H], dt.float32, tag="ew")
                        ew3 = ew[:].rearrange("p (b h) -> p b h", h=H)
                        nc.vector.tensor_tensor(
                            ew3, es3,
                            psD[:, 0:nb * H].rearrange("p (b h) -> p b h", h=H),
                            op=AL.add,
                        )
                        nc.vector.scalar_tensor_tensor(
                            ew[:], ew[:], 0.2, ew[:], op0=AL.mult, op1=AL.max
                        )
                        w16 = wp.tile([128, nb, H], dt.float16, tag="w16")
                        nc.scalar.activation(w16[:], ew3, AF.Exp, bias=-shift)
                        if l1:
                            # w16 is ready before the denses, so the drain
                            # IS the scale: one DVE op per chunk
                            for k in range(nb):
                                lhsT = xg[:, 0, k * 128:(k + 1) * 128]
                                psH = ps.tile(
                                    [128, 1024], dt.float32, tag="H", name="psH",
                                    bufs=2,
                                )
                                nc.tensor.matmul(
                                    psH[:, 0:512], lhsT, w1s[:, 0:512],
                                    start=True, stop=True,
                                )
                                nc.tensor.matmul(
                                    psH[:, 512:1024], lhsT, w1s[:, 512:1024],
                                    start=True, stop=True,
                                )
                                wk = w16[:, k, :].rearrange(
                                    "p (o h) -> p o h", o=1
                                ).to_broadcast([128, C, H])
                                nc.vector.tensor_tensor(
                                    g16[:, k, 0:DO].rearrange(
                                        "p (c h) -> p c h", h=H
                                    ),
                                    psH[:].rearrange("p (c h) -> p c h", h=H),
                                    wk, op=AL.mult,
                                )
                        else:
                            # scale rows per head in place (stride-0 middle)
                            for k in range(nb):
                                wk = w16[:, k, :].rearrange(
                                    "p (o h) -> p o h", o=1
                                ).to_broadcast([128, C, H])
                                nc.vector.tensor_tensor(
                                    g16[:, k, 0:DO].rearrange(
                                        "p (c h) -> p c h", h=H
                                    ),
                                    g16[:, k, 0:DO].rearrange(
                                        "p (c h) -> p c h", h=H
                                    ),
                                    wk, op=AL.mult,
                                )
                        if not sepC:
                            # denominator rides in psB: copy w into g16 tail
                            nc.vector.tensor_copy(
                                g16[:, 0:nb, DO:DO + H], w16[:]
                            )
                        # aggregation matmuls
                        for k in range(nb):
                            kk = k0 + k
                            fl, ll = kk == 0, kk == K_T - 1
                            sT = s[:, k * 128:(k + 1) * 128]
                            bw = min(512, NW - 512)
                            nc.tensor.matmul(
                                psA[:], sT, g16[:, k, 0:512], start=fl, stop=ll
                            )
                            nc.tensor.matmul(
                                psB[:, :bw], sT, g16[:, k, 512:512 + bw],
                                start=fl, stop=ll,
                            )
                            if sepC:
                                nc.tensor.matmul(
                                    psC[:, :H], sT, w16[:, k, :],
                                    start=fl, stop=ll,
                                )
                    epi_fn(t, psA, psB, psC)

            # ---------- epilogues ----------
            def normalize12(t, psA, psB, psC, H, C):
                dn = wp.tile([128, H], dt.float32, tag="dn")
                nc.vector.tensor_scalar_max(dn[:], psC[:, :H], 1e-16)
                r = wp.tile([128, H], dt.float32, tag="r")
                nc.vector.reciprocal(r[:], dn[:])
                xt = ep.tile([128, 1024], dt.float16, tag="xt")
                rb = r[:].rearrange("p (o h) -> p o h", o=1).to_broadcast(
                    [128, 128, H]
                )
                for half, pst in ((0, psA), (1, psB)):
                    nc.vector.tensor_tensor(
                        xt[:, half * 512:(half + 1) * 512].rearrange(
                            "p (c h) -> p c h", h=H
                        ),
                        pst[:].rearrange("p (c h) -> p c h", h=H),
                        rb, op=AL.mult,
                    )
                return xt

            def elu_into(xt, dest):
                # elu(x) = relu(x) + exp(-relu(-x)) - 1, keeping DVE light
                neg = ep.tile([128, 1024], dt.float16, tag="neg")
                nc.scalar.activation(neg[:], xt[:], AF.Relu, scale=-1.0)
                en = ep.tile([128, 1024], dt.float16, tag="en")
                nc.scalar.activation(en[:], neg[:], AF.Exp, scale=-1.0)
                a = ep.tile([128, 1024], dt.float16, tag="a")
                nc.scalar.activation(a[:], xt[:], AF.Relu)
                nc.vector.scalar_tensor_tensor(
                    dest, a[:], -1.0, en[:], op0=AL.add, op1=AL.add
                )

            def dense_next(t, xsrc, w_sb, DO, H, ESOFF, bounce, edlN, tableN):
                # PE-transpose x tile into xTs
                tp = ps.tile([128, 8, 128], dt.float16, tag="H", name="tp",
                             bufs=2)
                for fb in range(8):
                    nc.tensor.transpose(
                        tp[:, fb, :], xsrc[:, fb * 128:(fb + 1) * 128], idf16[:]
                    )
                xts = dp.tile([128, 8, 128], dt.float16, tag="xts")
                nc.scalar.copy(xts[:], tp[:])
                used = ESOFF + H
                tabst = ep.tile([128, used], dt.float16, tag="tabst", bufs=3)
                psE2 = ps.tile([128, 128], dt.float32, tag="DE", name="psE2")
                for k in range(8):
                    nc.tensor.matmul(
                        psE2[:, 64:64 + 2 * H],
                        xts[:, k, :],
                        w_sb[:, k, DO:DO + 2 * H],
                        start=(k == 0), stop=(k == 7),
                    )
                nc.vector.tensor_copy(
                    tabst[:, ESOFF:ESOFF + H], psE2[:, 64:64 + H]
                )
                nc.vector.tensor_copy(edlN[:, t, 0:H], psE2[:, 64 + H:64 + 2 * H])
                psN = ps.tile([128, 1024], dt.float32, tag="H", name="psN",
                              bufs=2)
                for si, (o, wd) in enumerate(((0, 512), (512, DO - 512))):
                    for k in range(8):
                        nc.tensor.matmul(
                            psN[:, o:o + wd],
                            xts[:, k, :],
                            w_sb[:, k, o:o + wd],
                            start=(k == 0), stop=(k == 7),
                        )
                    nc.scalar.copy(tabst[:, o:o + wd], psN[:, o:o + wd])
                nc.sync.dma_start(bounce[t * P:(t + 1) * P, 0:used], tabst[:])
                if t % 2 == 1:
                    nc.gpsimd.collective_compute(
                        "AllGather", mybir.AluOpType.bypass,
                        ins=[bounce[(t - 1) * P:(t + 1) * P, :].opt()],
                        outs=[tableN[(t - 1) * 1024:(t + 1) * 1024, :].opt()],
                        replica_groups=RG,
                    )

            # L1 edge: epilogue computes x2 (elu), stores to xres, builds L2 table
            def epi1(t, psA, psB, psC):
                xt = normalize12(t, psA, psB, psC, 4, 256)
                elu_into(xt, xres[:, t, :])
                dense_next(
                    t, xres[:, t, :], w2s, 1024, 4, L2_ESOFF,
                    bounceB, edlB, tableB,
                )

            edge_phase(True, None, 0, 0, 4, 256, SHIFTS[0], edl1, epi1)

            # L2 edge: epilogue x3 = elu(norm + x2), builds L3 table
            def epi2(t, psA, psB, psC):
                xt = normalize12(t, psA, psB, psC, 4, 256)
                nc.vector.tensor_add(xt[:], xt[:], xres[:, t, :])
                x3 = ep.tile([128, 1024], dt.float16, tag="x3")
                elu_into(xt, x3[:])
                dense_next(
                    t, x3[:], w3s, 726, 6, L3_ESOFF,
                    bounce3, edl3, table3,
                )

            edge_phase(False, tableB, L2_TBC, L2_ESOFF, 4, 256, SHIFTS[1], edlB, epi2)

            # L3 edge: final epilogue (mean over 6 heads, sigmoid)
            def epi3(t, psA, psB, psC):
                t732 = ep.tile([128, 732], dt.float32, tag="t732")
                nc.scalar.copy(t732[:, 0:512], psA[:])
                nc.vector.tensor_copy(t732[:, 512:732], psB[:, 0:220])
                dn = wp.tile([128, 6], dt.float32, tag="dn3")
                nc.vector.tensor_scalar_max(dn[:], t732[:, 726:732], 1e-16)
                r = wp.tile([128, 6], dt.float32, tag="r3")
                nc.vector.reciprocal(r[:], dn[:])
                r6 = wp.tile([128, 6], dt.float32, tag="r6")
                nc.vector.tensor_scalar_mul(r6[:], r[:], 1.0 / 6.0)
                tmp = ep.tile([128, 726], dt.float32, tag="tmp726")
                nc.vector.tensor_tensor(
                    tmp[:].rearrange("p (c h) -> p c h", h=6),
                    t732[:, 0:726].rearrange("p (c h) -> p c h", h=6),
                    r6[:].rearrange("p (o h) -> p o h", o=1).to_broadcast(
                        [128, 121, 6]
                    ),
                    op=AL.mult,
                )
                o121 = ep.tile([128, 121], dt.float32, tag="o121")
                nc.vector.reduce_sum(
                    o121[:], tmp[:].rearrange("p (c h) -> p c h", h=6),
                    mybir.AxisListType.X,
                )
                nc.scalar.activation(o121[:], o121[:], AF.Sigmoid)
                nc.sync.dma_start(out[t * P:(t + 1) * P, :], o121[:])

            edge_phase(False, table3, L3_TBC, L3_ESOFF, 6, 121, SHIFTS[2], edl3, epi3)

    nc.compile()
    return nc


def run(inputs, trace=False, tmpdir=None):
    from concourse.bass_utils import run_bass_kernel_spmd

    K_T, shared, percore = preprocess(inputs)
    key = K_T
    if key not in _CACHE:
        _CACHE[key] = build_program(K_T)
    nc = _CACHE[key]
    in_maps = [{**shared, **percore[c]} for c in range(NCORES)]
    if trace:
        import types

        try:
            import antenv.axon_hooks  # noqa: F401
        except ImportError:
            from trn_agent_boot.trn_boot import _ntff_profile_via_ctypes

            m = types.ModuleType("antenv.axon_hooks")
            hook = _ntff_profile_via_ctypes("/opt/axon/libaxon_pjrt.so")
            m.get_axon_ntff_profile_hook = lambda: hook
            sys.modules["antenv.axon_hooks"] = m
    try:
        res = run_bass_kernel_spmd(
            nc, in_maps, list(range(NCORES)), trace=trace, tmpdir=tmpdir
        )
    except Exception:
        # A crashed earlier run can leave the device wedged; one retry
        # normally clears it (nrt reopen).
        import time

        time.sleep(2)
        res = run_bass_kernel_spmd(
            nc, in_maps, list(range(NCORES)), trace=trace, tmpdir=tmpdir
        )
    outs = []
    for c in range(NCORES):
        outs.append(res.results[c]["out"][:NPC])
    full = np.concatenate(outs, 0).astype(np.float32)
    return full, res


def kernel(**inputs):
    full, _ = run(inputs)
    return full


# revision 23
# speedup vs baseline: 1.2375x; 1.1416x over previous
"""3-layer GAT on 8 Trainium2 NeuronCores (Bass/Tile).

Sharding: dst-node data parallel, 1250 nodes/core (padded 1280 = 10 tiles).

v5 vs the fp16 baseline:
- L1 has no replicated dense phase / 21 MB DRAM table: per-edge source
  features come from a 256-B transpose-mode dma_gather (<=768 idxs per
  instruction -- larger transpose gathers fault) out of a [10240 x 128] fp16
  x table, and h|e_src are computed per 128-edge chunk on the PE.  The chunk
  PSUM is drained unscaled to fp16 on ACT (frees the bank immediately), then
  scaled in place by the attention weights on DVE once e_dst arrives.
- L2/L3 tables store h in fp8e4 (e_src stays fp16 at a fixed byte offset),
  shrinking gather rows 2304 B -> 1280/768 B.  The DVE scale reads fp8 and
  writes the scaled fp16 rows the aggregation matmuls consume.  Host numpy
  validation: fp8 h/w quantization gives ~6e-3 final rel err vs 2e-2 gate.
- Tables are exchanged with per-tile-pair fp8 AllGathers (half the bytes of
  the fp16 baseline) that overlap the edge phase.

Feature columns are interleaved (c*H + h) so per-head scaling is a single
stride-0-middle-broadcast DVE multiply; weights are permuted on host.
Matmuls accumulate fp32 in PSUM; denominators ride as extra columns (L3) or
a separate tiny PSUM accumulation (L1/L2).
"""
import sys

sys.path.insert(0, "/opt/trn_rl_repo")

import numpy as np
import ml_dtypes

NCORES, N, NPC, NPAD, T, P = 8, 10000, 1250, 1280, 10, 128
R = 10240            # table rows (pair-major: see _trow)
GB = 9               # chunks per gather batch (L2/L3)
GB1 = 6              # chunks per transpose-gather batch (L1)
SHIFTS = (3.5, 1.25, 1.0)

F16 = np.float16
F8 = ml_dtypes.float8_e4m3

# table row width in fp16 elems (gather elem bytes must be mult of 256)
L2_TBC, L2_ESOFF = 1152, 1024   # 1024 f16 h | 4 f16 es | pad (2304 B)
L3_TBC, L3_ESOFF = 768, 726    # 726 f16 h | 6 f16 es | pad (1536 B)


def _trow(g):
    # Row layout matches the 2-tile AllGather concat: [tile-pair][rank][tile][p]
    q, r = g // NPC, g % NPC
    t, p = r // P, r % P
    return (t // 2) * 2048 + q * 256 + (t % 2) * P + p


def _wrap_idx(idx):
    """[n] -> [128, n//16] int16 (wrapped in 16 partitions, replicated 8x)."""
    blk = idx.astype(np.int16).reshape(-1, 16).T.copy()
    return np.tile(blk, (8, 1))


def preprocess(inputs):
    x = np.asarray(inputs["x"], np.float32)
    ei = np.asarray(inputs["edge_index"])
    src = np.concatenate([ei[0], np.arange(N)]).astype(np.int64)
    dst = np.concatenate([ei[1], np.arange(N)]).astype(np.int64)
    order = np.argsort(dst, kind="stable")
    src, dst = src[order], dst[order]

    # per-(core,tile) edge lists and uniform chunk grid
    per = []
    K_T = 0
    for c in range(NCORES):
        m = (dst >= c * NPC) & (dst < (c + 1) * NPC)
        s, d = src[m], dst[m] - c * NPC
        tiles = []
        for t in range(T):
            mt = (d >= t * P) & (d < (t + 1) * P)
            tiles.append((s[mt], d[mt] - t * P))
            K_T = max(K_T, (int(mt.sum()) + P - 1) // P)
        per.append(tiles)
    NCH = T * K_T

    gidx, sts, st2s, xlocs = [], [], [], []
    for c in range(NCORES):
        ss = np.zeros((T, K_T * P), np.int64)
        dd = np.zeros((T, K_T * P), np.int64)
        vv = np.zeros((T, K_T * P), bool)
        for t in range(T):
            s, d = per[c][t]
            n = len(s)
            ss[t, :n], dd[t, :n], vv[t, :n] = s, d, True
        rows = _trow(ss).reshape(-1)
        gidx.append(_wrap_idx(rows))
        S = np.zeros((T, K_T * P, P), np.float32)
        ar = np.arange(K_T * P)
        for t in range(T):
            sl = ar[vv[t]]
            S[t, sl, dd[t][vv[t]]] = 1.0
        S = S.reshape(NCH, P, P)
        sts.append(np.ascontiguousarray(
            S.transpose(1, 0, 2).reshape(P, NCH * P)).astype(F16))
        st2s.append(np.ascontiguousarray(
            S.transpose(2, 0, 1).reshape(P, NCH * P)).astype(F16))
        # local x transposed [128 feat x NPAD] (L1 local e_dst matmuls)
        xl = np.zeros((NPAD, 128), np.float32)
        xl[:NPC, :50] = x[c * NPC:(c + 1) * NPC]
        xlocs.append(np.ascontiguousarray(xl.T).astype(F16))

    # weights (shared), interleaved feature order (c*H + h)
    def w_aug(W, a_s, a_d, fin_pad, prev_hc=None):
        W = np.asarray(W, np.float32)
        H, C = a_s.shape
        F = W.shape[1]
        if prev_hc is not None:
            Hp, Cp = prev_hc
            perm = (np.arange(Cp)[:, None] + np.arange(Hp)[None, :] * Cp).reshape(-1)
            W = W[:, perm]
        Wp = W.reshape(H, C, F)
        Wi = np.transpose(Wp, (2, 1, 0)).reshape(F, C * H)
        es = np.einsum("hcf,hc->fh", Wp, np.asarray(a_s, np.float32))
        ed = np.einsum("hcf,hc->fh", Wp, np.asarray(a_d, np.float32))
        out = np.concatenate([Wi, es, ed], 1)
        return np.concatenate(
            [out, np.zeros((fin_pad - F, out.shape[1]), np.float32)], 0
        ).astype(F16)

    w1 = w_aug(inputs["W1"], np.asarray(inputs["as1"]), np.asarray(inputs["ad1"]), 128)
    w2 = w_aug(inputs["W2"], np.asarray(inputs["as2"]), np.asarray(inputs["ad2"]), 1024,
               prev_hc=(4, 256))
    w3 = w_aug(inputs["W3"], np.asarray(inputs["as3"]), np.asarray(inputs["ad3"]), 1024,
               prev_hc=(4, 256))

    # x table for L1 transpose-gather: [R, 128] fp16, row = _trow(node)
    xt = np.zeros((R, 128), np.float32)
    xt[_trow(np.arange(N)), :50] = x
    x1T = xt.astype(F16)

    shared = {"x1T": x1T, "w1": w1, "w2": w2, "w3": w3}
    percore = [
        {"gidx": gidx[c], "st": sts[c], "st2": st2s[c], "x1lsT": xlocs[c]}
        for c in range(NCORES)
    ]
    return K_T, shared, percore


_CACHE = {}


def build_program(K_T):
    import concourse.bacc as bacc
    import concourse.mybir as mybir
    import concourse.tile as tile

    dt = mybir.dt
    AF = mybir.ActivationFunctionType
    AL = mybir.AluOpType
    NCH = T * K_T

    nc = bacc.Bacc("TRN2", target_bir_lowering=False, debug=False, num_devices=NCORES)

    def register_const(val):
        t = nc.alloc_sbuf_tensor(f"constx-{val}", [128, 1], dt.float32)
        nc.gpsimd.memset(t.ap(), val)
        nc.const_aps.aps[(dt.float32, val)] = t.ap()

    for s in SHIFTS:
        if (dt.float32, -s) not in nc.const_aps.aps:
            register_const(-s)
    nc.all_engine_barrier()

    x1T = nc.dram_tensor("x1T", [R, 128], dt.float16, kind="ExternalInput")
    w1 = nc.dram_tensor("w1", [128, 1032], dt.float16, kind="ExternalInput")
    w2 = nc.dram_tensor("w2", [1024, 1032], dt.float16, kind="ExternalInput")
    w3 = nc.dram_tensor("w3", [1024, 738], dt.float16, kind="ExternalInput")
    gidx = nc.dram_tensor("gidx", [128, NCH * 8], dt.int16, kind="ExternalInput")
    st = nc.dram_tensor("st", [128, NCH * 128], dt.float16, kind="ExternalInput")
    st2 = nc.dram_tensor("st2", [128, NCH * 128], dt.float16, kind="ExternalInput")
    x1lsT = nc.dram_tensor("x1lsT", [128, NPAD], dt.float16, kind="ExternalInput")
    out = nc.dram_tensor("out", [NPAD, 121], dt.float32, kind="ExternalOutput")

    tableB = nc.dram_tensor("tableB", [R, L2_TBC], dt.float16, addr_space="Shared")
    table3 = nc.dram_tensor("table3", [R, L3_TBC], dt.float16, addr_space="Shared")
    bounceB = nc.dram_tensor("bounceB", [NPAD, L2_TBC], dt.float16)
    bounce3 = nc.dram_tensor("bounce3", [NPAD, L3_TBC], dt.float16)

    RG = [list(range(NCORES))]

    with tile.TileContext(nc) as tc:
        from concourse.masks import make_identity

        with (
            tc.tile_pool(name="per", bufs=1) as per,
            tc.tile_pool(name="gp", bufs=2) as gp,
            tc.tile_pool(name="dp", bufs=2) as dp,
            tc.tile_pool(name="sp", bufs=2) as sp,
            tc.tile_pool(name="wp", bufs=2) as wp,
            tc.tile_pool(name="ep", bufs=2) as ep,
            tc.tile_pool(name="ps", bufs=1, space="PSUM") as ps,
        ):
            # persistent loads
            w1s = per.tile([128, 1032], dt.float16)
            nc.sync.dma_start(w1s[:], w1[:])
            w2s = per.tile([128, 8, 1032], dt.float16)
            nc.sync.dma_start(w2s[:], w2.ap().rearrange("(a p) n -> p a n", p=128))
            w3s = per.tile([128, 8, 738], dt.float16)
            nc.sync.dma_start(w3s[:], w3.ap().rearrange("(a p) n -> p a n", p=128))
            g1i = per.tile([128, NCH * 8], dt.int16)
            nc.sync.dma_start(g1i[:], gidx[:])
            x1ls = per.tile([128, NPAD], dt.float16)
            nc.sync.dma_start(x1ls[:], x1lsT[:])
            edl1 = per.tile([128, T, 8], dt.float16)
            edlB = per.tile([128, T, 8], dt.float16)
            edl3 = per.tile([128, T, 8], dt.float16)
            idf16 = per.tile([128, 128], dt.float16)
            make_identity(nc, idf16[:])
            xres = per.tile([128, T, 1024], dt.float16)

            # L1 local e_dst (tiny matmuls from local x)
            for t in range(T):
                pse = ps.tile([128, 128], dt.float32, tag="DE", name="pse")
                nc.tensor.matmul(
                    pse[:, 64:72], x1ls[:, t * P:(t + 1) * P], w1s[:, 1024:1032],
                    start=True, stop=True,
                )
                nc.scalar.copy(edl1[:, t, 0:4], pse[:, 68:72])

            # ---------- uniform edge phase ----------
            def edge_phase(l1, table, TBC, ESOFF, H, C, shift, edl, epi_fn):
                DO = H * C
                NW = DO + H          # with-denominator width (L3 rides in psB)
                sepC = NW > 1024     # L1/L2: denominator in separate psC
                for t in range(T):
                    psA = ps.tile([128, 512], dt.float32, tag="A")
                    psB = ps.tile([128, 512], dt.float32, tag="B")
                    if sepC:
                        psC = ps.tile([128, 64], dt.float32, tag="C")
                    else:
                        psC = None
                    if l1:
                        batches = [(k, min(k + GB1, K_T)) for k in range(0, K_T, GB1)]
                    else:
                        batches = [(0, GB), (GB, K_T)]
                    for (k0, k1) in batches:
                        nb = k1 - k0
                        off8 = (t * K_T + k0) * 8
                        c0, c1 = (t * K_T + k0) * 128, (t * K_T + k1) * 128
                        s = sp.tile([128, nb * 128], dt.float16, tag="s",
                                    bufs=3)
                        nc.sync.dma_start(s[:], st[:, c0:c1])
                        s2 = dp.tile([128, nb * 128], dt.float16, tag="s2",
                                     bufs=3)
                        nc.sync.dma_start(s2[:], st2[:, c0:c1])
                        if l1:
                            xg = gp.tile([128, 1, nb * 128], dt.float16,
                                         tag="xg", bufs=3)
                            nc.gpsimd.dma_gather(
                                xg[:], x1T.ap(), g1i[:, off8:off8 + nb * 8],
                                num_idxs=nb * 128, num_idxs_reg=nb * 128,
                                elem_size=128, transpose=True,
                            )
                            g16 = gp.tile([128, GB1, 1024], dt.float16,
                                          tag="g", bufs=4)
                            de = ps.tile([128, 128], dt.float32, tag="DE", name="de")
                            # e_src pass first so the w chain never waits on
                            # the h denses below
                            for k in range(nb):
                                nc.tensor.matmul(
                                    de[:, 64 + k * 4:64 + k * 4 + 4],
                                    xg[:, 0, k * 128:(k + 1) * 128],
                                    w1s[:, 1024:1028], start=True, stop=True,
                                )
                            est = wp.tile([128, nb, 4], dt.float16, tag="est")
                            nc.vector.tensor_copy(
                                est[:],
                                de[:, 64:64 + nb * 4].rearrange(
                                    "p (b h) -> p b h", h=4
                                ),
                            )
                            es3 = est[:]
                        else:
                            g16 = gp.tile([128, GB, TBC], dt.float16, tag="g",
                                          bufs=4)
                            nc.gpsimd.dma_gather(
                                g16[:, 0:nb], table.ap(),
                                g1i[:, off8:off8 + nb * 8],
                                num_idxs=nb * 128, num_idxs_reg=nb * 128,
                                elem_size=TBC, single_packet=False,
                            )
                            es3 = g16[:, 0:nb, ESOFF:ESOFF + H]
                        # e_dst broadcast to edges
                        if l1:
                            psD = de
                        else:
                            psD = ps.tile([128, 128], dt.float32, tag="DE",
                                          name="psD")
                        for k in range(nb):
                            nc.tensor.matmul(
                                psD[:, k * H:(k + 1) * H],
                                s2[:, k * 128:(k + 1) * 128],
                                edl[:, t, 0:H],
                                start=True, stop=True,
                            )
                        # w = exp(leaky(es + ed) - shift)
                        ew = wp.tile([128, nb * H], dt.float32, tag="ew")
                        ew3 = ew[:].rearrange("p (b h) -> p b h", h=H)
                        nc.vector.tensor_tensor(
                            ew3, es3,
                            psD[:, 0:nb * H].rearrange("p (b h) -> p b h", h=H),
                            op=AL.add,
                        )
                        nc.vector.scalar_tensor_tensor(
                            ew[:], ew[:], 0.2, ew[:], op0=AL.mult, op1=AL.max
                        )
                        w16 = wp.tile([128, nb, H], dt.float16, tag="w16")
                        nc.scalar.activation(w16[:], ew3, AF.Exp, bias=-shift)
                        if l1:
                            # w16 is ready before the denses, so the drain
                            # IS the scale: one DVE op per chunk
                            for k in range(nb):
                                lhsT = xg[:, 0, k * 128:(k + 1) * 128]
                                psH = ps.tile(
                                    [128, 1024], dt.float32, tag="H", name="psH",
                                    bufs=2,
                                )
                                nc.tensor.matmul(
                                    psH[:, 0:512], lhsT, w1s[:, 0:512],
                                    start=True, stop=True,
                                )
                                nc.tensor.matmul(
                                    psH[:, 512:1024], lhsT, w1s[:, 512:1024],
                                    start=True, stop=True,
                                )
                                wk = w16[:, k, :].rearrange(
                                    "p (o h) -> p o h", o=1
                                ).to_broadcast([128, C, H])
                                nc.vector.tensor_tensor(
                                    g16[:, k, 0:DO].rearrange(
                                        "p (c h) -> p c h", h=H
                                    ),
                                    psH[:].rearrange("p (c h) -> p c h", h=H),
                                    wk, op=AL.mult,
                                )
                        else:
                            # scale rows per head in place (stride-0 middle)
                            for k in range(nb):
                                wk = w16[:, k, :].rearrange(
                                    "p (o h) -> p o h", o=1
                                ).to_broadcast([128, C, H])
                                nc.vector.tensor_tensor(
                                    g16[:, k, 0:DO].rearrange(
                                        "p (c h) -> p c h", h=H
                                    ),
                                    g16[:, k, 0:DO].rearrange(
                                        "p (c h) -> p c h", h=H
                                    ),
                                    wk, op=AL.mult,
                                )
                        if not sepC:
                            # denominator rides in psB: copy w into g16 tail
                            nc.vector.tensor_copy(
                                g16[:, 0:nb, DO:DO + H], w16[:]
                            )
                        # aggregation matmuls
                        for k in range(nb):
                            kk = k0 + k
                            fl, ll = kk == 0, kk == K_T - 1
                            sT = s[:, k * 128:(k + 1) * 128]
                            bw = min(512, NW - 512)
                            nc.tensor.matmul(
                                psA[:], sT, g16[:, k, 0:512], start=fl, stop=ll
                            )
                            nc.tensor.matmul(
                                psB[:, :bw], sT, g16[:, k, 512:512 + bw],
                                start=fl, stop=ll,
                            )
                            if sepC:
                                nc.tensor.matmul(
                                    psC[:, :H], sT, w16[:, k, :],
                                    start=fl, stop=ll,
                                )
                    epi_fn(t, psA, psB, psC)

            # ---------- epilogues ----------
            def normalize12(t, psA, psB, psC, H, C):
                dn = wp.tile([128, H], dt.float32, tag="dn")
                nc.vector.tensor_scalar_max(dn[:], psC[:, :H], 1e-16)
                r = wp.tile([128, H], dt.float32, tag="r")
                nc.vector.reciprocal(r[:], dn[:])
                xt = ep.tile([128, 1024], dt.float16, tag="xt")
                rb = r[:].rearrange("p (o h) -> p o h", o=1).to_broadcast(
                    [128, 128, H]
                )
                for half, pst in ((0, psA), (1, psB)):
                    nc.vector.tensor_tensor(
                        xt[:, half * 512:(half + 1) * 512].rearrange(
                            "p (c h) -> p c h", h=H
                        ),
                        pst[:].rearrange("p (c h) -> p c h", h=H),
                        rb, op=AL.mult,
                    )
                return xt

            def elu_into(xt, dest):
                # elu(x) = relu(x) + exp(-relu(-x)) - 1, keeping DVE light
                neg = ep.tile([128, 1024], dt.float16, tag="neg")
                nc.scalar.activation(neg[:], xt[:], AF.Relu, scale=-1.0)
                en = ep.tile([128, 1024], dt.float16, tag="en")
                nc.scalar.activation(en[:], neg[:], AF.Exp, scale=-1.0)
                a = ep.tile([128, 1024], dt.float16, tag="a")
                nc.scalar.activation(a[:], xt[:], AF.Relu)
                nc.vector.scalar_tensor_tensor(
                    dest, a[:], -1.0, en[:], op0=AL.add, op1=AL.add
                )

            def dense_next(t, xsrc, w_sb, DO, H, ESOFF, bounce, edlN, tableN):
                # PE-transpose x tile into xTs
                tp = ps.tile([128, 8, 128], dt.float16, tag="H", name="tp",
                             bufs=2)
                for fb in range(8):
                    nc.tensor.transpose(
                        tp[:, fb, :], xsrc[:, fb * 128:(fb + 1) * 128], idf16[:]
                    )
                xts = dp.tile([128, 8, 128], dt.float16, tag="xts")
                nc.scalar.copy(xts[:], tp[:])
                used = ESOFF + H
                tabst = ep.tile([128, used], dt.float16, tag="tabst", bufs=3)
                psE2 = ps.tile([128, 128], dt.float32, tag="DE", name="psE2")
                for k in range(8):
                    nc.tensor.matmul(
                        psE2[:, 64:64 + 2 * H],
                        xts[:, k, :],
                        w_sb[:, k, DO:DO + 2 * H],
                        start=(k == 0), stop=(k == 7),
                    )
                nc.vector.tensor_copy(
                    tabst[:, ESOFF:ESOFF + H], psE2[:, 64:64 + H]
                )
                nc.vector.tensor_copy(edlN[:, t, 0:H], psE2[:, 64 + H:64 + 2 * H])
                psN = ps.tile([128, 1024], dt.float32, tag="H", name="psN",
                              bufs=2)
                for si, (o, wd) in enumerate(((0, 512), (512, DO - 512))):
                    for k in range(8):
                        nc.tensor.matmul(
                            psN[:, o:o + wd],
                            xts[:, k, :],
                            w_sb[:, k, o:o + wd],
                            start=(k == 0), stop=(k == 7),
                        )
                    nc.scalar.copy(tabst[:, o:o + wd], psN[:, o:o + wd])
                nc.sync.dma_start(bounce[t * P:(t + 1) * P, 0:used], tabst[:])
                if t % 2 == 1:
                    nc.gpsimd.collective_compute(
                        "AllGather", mybir.AluOpType.bypass,
                        ins=[bounce[(t - 1) * P:(t + 1) * P, :].opt()],
                        outs=[tableN[(t - 1) * 1024:(t + 1) * 1024, :].opt()],
                        replica_groups=RG,
                    )

            # L1 edge: epilogue computes x2 (elu), stores to xres, builds L2 table
            def epi1(t, psA, psB, psC):
                xt = normalize12(t, psA, psB, psC, 4, 256)
                elu_into(xt, xres[:, t, :])
                dense_next(
                    t, xres[:, t, :], w2s, 1024, 4, L2_ESOFF,
                    bounceB, edlB, tableB,
                )

            edge_phase(True, None, 0, 0, 4, 256, SHIFTS[0], edl1, epi1)

            # L2 edge: epilogue x3 = elu(norm + x2), builds L3 table
            def epi2(t, psA, psB, psC):
                xt = normalize12(t, psA, psB, psC, 4, 256)
                nc.vector.tensor_add(xt[:], xt[:], xres[:, t, :])
                x3 = ep.tile([128, 1024], dt.float16, tag="x3")
                elu_into(xt, x3[:])
                dense_next(
                    t, x3[:], w3s, 726, 6, L3_ESOFF,
                    bounce3, edl3, table3,
                )

            edge_phase(False, tableB, L2_TBC, L2_ESOFF, 4, 256, SHIFTS[1], edlB, epi2)

            # L3 edge: final epilogue (mean over 6 heads, sigmoid)
            def epi3(t, psA, psB, psC):
                t732 = ep.tile([128, 732], dt.float32, tag="t732")
                nc.scalar.copy(t732[:, 0:512], psA[:])
                nc.vector.tensor_copy(t732[:, 512:732], psB[:, 0:220])
                dn = wp.tile([128, 6], dt.float32, tag="dn3")
                nc.vector.tensor_scalar_max(dn[:], t732[:, 726:732], 1e-16)
                r = wp.tile([128, 6], dt.float32, tag="r3")
                nc.vector.reciprocal(r[:], dn[:])
                r6 = wp.tile([128, 6], dt.float32, tag="r6")
                nc.vector.tensor_scalar_mul(r6[:], r[:], 1.0 / 6.0)
                tmp = ep.tile([128, 726], dt.float32, tag="tmp726")
                nc.vector.tensor_tensor(
                    tmp[:].rearrange("p (c h) -> p c h", h=6),
                    t732[:, 0:726].rearrange("p (c h) -> p c h", h=6),
                    r6[:].rearrange("p (o h) -> p o h", o=1).to_broadcast(
                        [128, 121, 6]
                    ),
                    op=AL.mult,
                )
                o121 = ep.tile([128, 121], dt.float32, tag="o121")
                nc.vector.reduce_sum(
                    o121[:], tmp[:].rearrange("p (c h) -> p c h", h=6),
                    mybir.AxisListType.X,
                )
                nc.scalar.activation(o121[:], o121[:], AF.Sigmoid)
                nc.sync.dma_start(out[t * P:(t + 1) * P, :], o121[:])

            edge_phase(False, table3, L3_TBC, L3_ESOFF, 6, 121, SHIFTS[2], edl3, epi3)

    nc.compile()
    return nc


def run(inputs, trace=False, tmpdir=None):
    from concourse.bass_utils import run_bass_kernel_spmd

    K_T, shared, percore = preprocess(inputs)
    key = K_T
    if key not in _CACHE:
        _CACHE[key] = build_program(K_T)
    nc = _CACHE[key]
    in_maps = [{**shared, **percore[c]} for c in range(NCORES)]
    if trace:
        import types

        try:
            import antenv.axon_hooks  # noqa: F401
        except ImportError:
            from trn_agent_boot.trn_boot import _ntff_profile_via_ctypes

            m = types.ModuleType("antenv.axon_hooks")
            hook = _ntff_profile_via_ctypes("/opt/axon/libaxon_pjrt.so")
            m.get_axon_ntff_profile_hook = lambda: hook
            sys.modules["antenv.axon_hooks"] = m
    try:
        res = run_bass_kernel_spmd(
            nc, in_maps, list(range(NCORES)), trace=trace, tmpdir=tmpdir
        )
    except Exception:
        # A crashed earlier run can leave the device wedged; one retry
        # normally clears it (nrt reopen).
        import time

        time.sleep(2)
        res = run_bass_kernel_spmd(
            nc, in_maps, list(range(NCORES)), trace=trace, tmpdir=tmpdir
        )
    outs = []
    for c in range(NCORES):
        outs.append(res.results[c]["out"][:NPC])
    full = np.concatenate(outs, 0).astype(np.float32)
    return full, res


def kernel(**inputs):
    full, _ = run(inputs)
    return full
